# revision 1
# baseline (speedup 1.0000x reference)
"""Trainium2 Bass kernel for nn_Net_72447508349482 (dense_cnn).

Pipeline per core (batch sharded 256 -> 8 x 32):
  conv1 (PE matmul over im2col) -> maxpool2 (DVE) -> adder2d
  (|patch - w| on DVE tensor_scalar(add, abs_max) + ACT activation(Abs, bias),
   partition-reduce over k via PE one-hot matmul accumulating in PSUM)
  -> minpool (= -maxpool(-x), sign folded into BN affine) -> BN batch stats
  with cross-core AllReduce of [50,2] sums -> FC1 + ReLU -> FC2 -> log_softmax.

conv1 bias is folded into the adder weights on the host:
  maxpool(conv+b) = maxpool(conv)+b and |p + b - w| = |p - (w - b)|.

Self-contained: includes the BIR multi-wait splitting fix required by this
container's walrus build (rejects >1 sync wait per instruction).
"""

import json

import numpy as np

import concourse.bass as bass
import concourse.mybir as mybir
import concourse.tile as tile
from concourse.bass_utils import run_bass_kernel_spmd

N_CORES = 8
B_TOTAL = 256
BC = B_TOTAL // N_CORES          # 32 per core
BN_EPS = 1e-5
F32 = mybir.dt.float32
BF16 = mybir.dt.bfloat16
ALU = mybir.AluOpType
ACTF = mybir.ActivationFunctionType
F32R = mybir.dt.float32r


def _r(ap):
    """Bitcast to float32r: PE runs 1 cycle/row (vs 4 for plain fp32)."""
    return ap.bitcast(F32R)

# adder engine split (bf16 DVE at 4x): 11 channels on ACT, 39 on DVE
ACT_O = {0, 4, 9, 13, 18, 22, 27, 31, 36, 40, 45}


# --------------------------------------------------------------------------
# BIR post-processing: split multi-wait instructions (walrus limit = 1).
# --------------------------------------------------------------------------
def _fix_bir_bytes(raw: bytes) -> bytes:
    d = json.loads(raw)
    for fn in d.get("functions", []):
        for b in fn.get("blocks", []):
            insts = b.get("instructions", [])
            i = 0
            while i < len(insts):
                ins = insts[i]
                si = ins.get("sync_info") or {}
                w = si.get("on_wait") or []
                if len(w) > 1:
                    for j, extra in enumerate(w[:-1]):
                        insts.insert(i, {
                            "name": f"{ins['name']}_wsplit{j}",
                            "opcode": "EventSemaphore",
                            "engine": ins["engine"],
                            "ins": [], "outs": [],
                            "debug": ins.get("debug", 0),
                            "sync_info": {"on_update": [], "on_wait": [extra]},
                        })
                        i += 1
                    si["on_wait"] = w[-1:]
                i += 1
    return json.dumps(d).encode()


def _patch_nc(nc):
    cls_fn = type(nc).to_json_bytes
    nc.to_json_bytes = lambda: _fix_bir_bytes(cls_fn(nc))
    return nc


# --------------------------------------------------------------------------
# Host-side input prep (pure rearranges/folds of the given parameters).
# --------------------------------------------------------------------------
def host_prep(inputs):
    f = lambda k: np.ascontiguousarray(np.asarray(inputs[k], np.float32))
    x = f("x")                                       # [256,1,28,28]
    conv1_w, conv1_b, adder_w = f("conv1_w"), f("conv1_b"), f("adder_w")
    p = {}
    p["x_im"] = x.reshape(B_TOTAL, 784)
    p["w1conv"] = None  # set below (bf16)
    wfold = adder_w - conv1_b[None, :, None, None]
    A = (-wfold.reshape(50, 20, 25))                 # [o, c, m]
    # chunk q: channels 5q..5q+4; within-chunk partition p = m*5 + c'
    negwT = np.stack(
        [A[:, 5 * q:5 * q + 5, :].transpose(2, 1, 0).reshape(125, 50)
         for q in range(4)], axis=1)                 # [125, 4, 50]
    p["negwT"] = np.ascontiguousarray(negwT)
    poswT = np.stack(
        [(-A)[:, 5 * q:5 * q + 5, :].transpose(2, 1, 0).reshape(125, 50)
         for q in range(4)], axis=1)                 # [125, 4, 50] = +w'
    p["poswT"] = np.ascontiguousarray(poswT)
    import ml_dtypes
    bf = ml_dtypes.bfloat16
    p["w1conv"] = np.ascontiguousarray(conv1_w.reshape(20, 25).T).astype(bf)
    oh = np.zeros((125, 100), bf)
    oh[:, 50] = bf(1.0)
    p["onehot"] = oh
    z2 = np.zeros((125, 100), bf)
    z2[:, 50] = bf(-2.0)
    p["z2"] = z2
    ones_dve = np.zeros((125, 50), bf)
    for o in range(50):
        if o not in ACT_O:
            ones_dve[:, o] = bf(1.0)
    p["ones_dve"] = ones_dve
    p["w1t"] = np.ascontiguousarray(
        f("fc1_w").reshape(500, 50, 16).transpose(1, 0, 2)).astype(bf)  # [50,500,16]
    p["b1t"] = np.ascontiguousarray(f("fc1_b").reshape(4, 125).T)       # [125,4]
    p["w2t"] = np.ascontiguousarray(
        f("fc2_w").T.reshape(4, 125, 10).transpose(1, 0, 2)).astype(bf)  # [125,4,10]
    p["b2"] = f("fc2_b").reshape(10, 1)
    p["neg_gamma"] = np.ascontiguousarray(-f("bn_gamma").reshape(50, 1))
    p["beta"] = f("bn_beta").reshape(50, 1)
    p["ident10"] = np.eye(10, dtype=np.float32)
    return p


# --------------------------------------------------------------------------
# Device program.
# --------------------------------------------------------------------------
def build_nc(reps: int = 1, for_sim: bool = False):
    nc = bass.Bass("TRN2", target_bir_lowering=False, debug=False,
                   num_devices=1 if for_sim else N_CORES)
    d_x = nc.dram_tensor("x_imT", [784, BC], BF16, kind="ExternalInput")
    d_w1conv = nc.dram_tensor("w1conv", [25, 20], BF16, kind="ExternalInput")
    d_negwT = nc.dram_tensor("negwT", [125, 4, 50], F32, kind="ExternalInput")
    d_onehot = nc.dram_tensor("onehot", [125, 100], BF16, kind="ExternalInput")
    d_poswT = nc.dram_tensor("poswT", [125, 4, 50], F32, kind="ExternalInput")
    d_z2 = nc.dram_tensor("z2", [125, 100], BF16, kind="ExternalInput")
    d_onesdve = nc.dram_tensor("ones_dve", [125, 50], BF16, kind="ExternalInput")
    d_w1t = nc.dram_tensor("w1t", [50, 500, 16], BF16, kind="ExternalInput")
    d_b1t = nc.dram_tensor("b1t", [125, 4], F32, kind="ExternalInput")
    d_w2t = nc.dram_tensor("w2t", [125, 4, 10], BF16, kind="ExternalInput")
    d_b2 = nc.dram_tensor("b2", [10, 1], F32, kind="ExternalInput")
    d_ng = nc.dram_tensor("neg_gamma", [50, 1], F32, kind="ExternalInput")
    d_beta = nc.dram_tensor("beta", [50, 1], F32, kind="ExternalInput")
    d_id10 = nc.dram_tensor("ident10", [10, 10], F32, kind="ExternalInput")
    d_y = nc.dram_tensor("y", [BC, 10], F32, kind="ExternalOutput")
    cc_in = nc.dram_tensor("cc_in", [50, 2], F32)
    cc_out = nc.dram_tensor("cc_out", [50, 2], F32, addr_space="Shared")

    with tile.TileContext(nc) as tc:
        with (
            tc.tile_pool(name="consts", bufs=1) as consts,
            tc.tile_pool(name="big", bufs=1) as big,
            tc.tile_pool(name="work", bufs=3) as work,
            tc.tile_pool(name="absp", bufs=16) as absp,
            tc.tile_pool(name="small", bufs=1) as small,
        ):
            # ---- load constants ----
            s_w1conv = consts.tile([25, 20], BF16)
            nc.sync.dma_start(out=s_w1conv, in_=d_w1conv[:, :])
            s_negwT = consts.tile([125, 4, 50], F32)
            nc.sync.dma_start(out=s_negwT, in_=d_negwT[:, :, :])
            s_onehot = consts.tile([125, 100], BF16)
            nc.sync.dma_start(out=s_onehot, in_=d_onehot[:, :])
            s_poswT = consts.tile([125, 4, 50], F32)
            nc.sync.dma_start(out=s_poswT, in_=d_poswT[:, :, :])
            s_z2 = consts.tile([125, 100], BF16)
            nc.sync.dma_start(out=s_z2, in_=d_z2[:, :])
            s_onesdve = consts.tile([125, 50], BF16)
            nc.sync.dma_start(out=s_onesdve, in_=d_onesdve[:, :])
            s_w1t = consts.tile([50, 500, 16], BF16)
            nc.sync.dma_start(out=s_w1t, in_=d_w1t[:, :, :])
            s_b1t = consts.tile([125, 4], F32)
            nc.sync.dma_start(out=s_b1t, in_=d_b1t[:, :])
            s_w2t = consts.tile([125, 4, 10], BF16)
            nc.sync.dma_start(out=s_w2t, in_=d_w2t[:, :, :])
            s_b2 = consts.tile([10, 1], F32)
            nc.sync.dma_start(out=s_b2, in_=d_b2[:, :])
            s_ng = consts.tile([50, 1], F32)
            nc.sync.dma_start(out=s_ng, in_=d_ng[:, :])
            s_beta = consts.tile([50, 1], F32)
            nc.sync.dma_start(out=s_beta, in_=d_beta[:, :])
            s_id10 = consts.tile([10, 10], F32)
            nc.sync.dma_start(out=s_id10, in_=d_id10[:, :])

            # ---- conv1 + pool1 -> h1 [20, 12, 12, 32] (i, j, b layout) ----
            x_t = d_x.ap().tensor
            for _rep in range(reps):
              h1 = big.tile([20, 12, 12, BC], BF16, tag="h1", name="h1")
              with tc.tile_pool(name="convps", bufs=2, space="PSUM") as convps:
                  for ic in range(12):                 # chunks of 2 output rows
                      i0 = 2 * ic
                      rhs = work.tile([25, 2, 24, BC], BF16, tag="convrhs")
                      for kh in range(5):
                          src = bass.AP(
                              tensor=x_t,
                              offset=(i0 + kh) * 28 * BC,
                              ap=[[BC, 5], [28 * BC, 2], [1, 24 * BC]],
                          )
                          eng = nc.sync if kh % 2 == 0 else nc.scalar
                          eng.dma_start(out=rhs[kh * 5:kh * 5 + 5], in_=src)
                      ps = convps.tile([20, 2, 24, BC], F32, tag="convps")
                      rflat = rhs.rearrange("p a b c -> p (a b c)")
                      pflat = ps.rearrange("p a b c -> p (a b c)")
                      for s0, s1 in ((0, 512), (512, 1024), (1024, 1536)):
                          nc.tensor.matmul(pflat[:, s0:s1], s_w1conv[:, :],
                                           rflat[:, s0:s1], start=True,
                                           stop=True)
                      # pool1: copy only the odd-j half to SBUF (DVE may read
                      # one PSUM operand directly); alternate copy engine so
                      # ACT is free to start adder Abs ops sooner
                      codd = work.tile([20, 2, 12, BC], F32, tag="convodd")
                      pv = ps.rearrange("p a (j two) b -> p a j two b", two=2)
                      if ic % 2 == 0:
                          nc.scalar.activation(out=codd, in_=pv[:, :, :, 1, :],
                                               func=ACTF.Copy)
                      else:
                          nc.vector.tensor_copy(out=codd, in_=pv[:, :, :, 1, :])
                      h1j = work.tile([20, 2, 12, BC], F32, tag="h1j")
                      nc.vector.tensor_tensor(out=h1j, in0=pv[:, :, :, 0, :],
                                              in1=codd, op=ALU.max)
                      nc.vector.tensor_tensor(out=h1[:, ic], in0=h1j[:, 0],
                                              in1=h1j[:, 1], op=ALU.max)

              # ---- patches [125, 8, 8, 32] x 4 chunks; p = (kh*5+kw)*5 + c' ----
              patches = [big.tile([125, 8, 8, BC], BF16, tag=f"patches{q}",
                                  name=f"patches{q}") for q in range(4)]
              for q in range(4):
                  for kh in range(5):
                      for kw in range(5):
                          m = kh * 5 + kw
                          eng = nc.sync if m % 2 == 0 else nc.scalar
                          eng.dma_start(
                              out=patches[q][m * 5:m * 5 + 5],
                              in_=h1[5 * q:5 * q + 5, kh:kh + 8, kw:kw + 8, :],
                          )

              # ---- adder ----
              # ACT rows o: psum[o] += sum_k |patch_k + negw_k|  (one-hot +1)
              # DVE rows o: psum[o] += sum_k patch_k - 2*sum_k min(patch_k, w_k)
              #   (identity |a-b| = a + b - 2 min(a,b); the per-channel constant
              #    sum_k w_k is dropped -- BatchNorm cancels channel shifts)
              with tc.tile_pool(name="addps", bufs=1, space="PSUM") as addps:
                  psum_add = addps.tile([50, 2048], F32)
                  for q in range(4):
                      pq = patches[q].rearrange("p i j b -> p (i j b)")
                      # sum_k patch_k into all DVE rows
                      for nb in range(4):
                          nc.tensor.matmul(
                              psum_add[:, nb * 512:(nb + 1) * 512],
                              s_onesdve[:, :],
                              pq[:, nb * 512:(nb + 1) * 512],
                              start=(q == 0), stop=False)
                      dve_os = [o for o in range(50) if o not in ACT_O]
                      act_os = [o for o in range(50) if o in ACT_O]
                      order = []
                      di = ai = 0
                      for o_i in range(50):
                          # ~3.5 DVE per ACT, DVE first
                          if ai * 39 < di * 11 and ai < len(act_os):
                              order.append(act_os[ai]); ai += 1
                          elif di < len(dve_os):
                              order.append(dve_os[di]); di += 1
                          else:
                              order.append(act_os[ai]); ai += 1
                      for o in order:
                          a = absp.tile([125, 2048], BF16, tag="abs")
                          if o in ACT_O:
                              wv = s_negwT[:, q, o:o + 1]
                              nc.scalar.activation(out=a, in_=pq, func=ACTF.Abs,
                                                   bias=wv, scale=1.0)
                              lhsT = s_onehot[:, 50 - o:100 - o]
                          else:
                              wv = s_poswT[:, q, o:o + 1]
                              nc.vector.tensor_scalar(
                                  out=a, in0=pq, scalar1=wv, scalar2=None,
                                  op0=ALU.min)
                              lhsT = s_z2[:, 50 - o:100 - o]
                          last = (q == 3 and o == order[-1])
                          for nb in range(4):
                              nc.tensor.matmul(
                                  psum_add[:, nb * 512:(nb + 1) * 512],
                                  lhsT, a[:, nb * 512:(nb + 1) * 512],
                                  start=False, stop=last)

                  # ---- pool2 (min) -> h2m [50, 4, 4, 32] ----
                  hs = small.tile([50, 2048], F32)
                  nc.scalar.activation(out=hs, in_=psum_add, func=ACTF.Copy)
                  pv = hs.rearrange("p (i j two b) -> p i j two b",
                                    i=8, j=4, two=2)
                  h2j = small.tile([50, 8, 4, BC], F32)
                  nc.vector.tensor_tensor(out=h2j, in0=pv[:, :, :, 0, :],
                                          in1=pv[:, :, :, 1, :], op=ALU.min)
                  h2v = h2j.rearrange("p (i two) j b -> p i two j b", two=2)
                  h2m = small.tile([50, 4, 4, BC], F32)
                  nc.vector.tensor_tensor(out=h2m, in0=h2v[:, :, 0],
                                          in1=h2v[:, :, 1], op=ALU.min)

              h2f = h2m.rearrange("p i j b -> p (i j b)")      # [50, 512]

              # ---- BN stats + AllReduce ----
              stats = small.tile([50, 2], F32)
              nc.vector.tensor_reduce(out=stats[:, 0:1], in_=h2f,
                                      axis=mybir.AxisListType.X, op=ALU.add)
              junk = small.tile([50, 512], F32)
              nc.vector.tensor_tensor(out=junk, in0=h2f, in1=h2f, op=ALU.mult)
              nc.vector.tensor_reduce(out=stats[:, 1:2], in_=junk,
                                      axis=mybir.AxisListType.X, op=ALU.add)
              nc.sync.dma_start(out=cc_in[:, :], in_=stats)
              if not for_sim:
                  nc.gpsimd.collective_compute(
                      "AllReduce", ALU.add,
                      replica_groups=[list(range(N_CORES))],
                      ins=[cc_in.ap().opt()], outs=[cc_out.ap().opt()])
              gs = small.tile([50, 2], F32)
              nc.sync.dma_start(out=gs, in_=(cc_in if for_sim else cc_out)[:, :])

              # ---- BN affine coefficients ----
              inv_n = 1.0 / (B_TOTAL * 16)
              mean = small.tile([50, 1], F32)
              nc.vector.tensor_scalar(out=mean, in0=gs[:, 0:1], scalar1=inv_n,
                                      scalar2=None, op0=ALU.mult)
              e2 = small.tile([50, 1], F32)
              nc.vector.tensor_scalar(out=e2, in0=gs[:, 1:2], scalar1=inv_n,
                                      scalar2=None, op0=ALU.mult)
              msq = small.tile([50, 1], F32)
              nc.vector.tensor_tensor(out=msq, in0=mean, in1=mean, op=ALU.mult)
              var = small.tile([50, 1], F32)
              nc.vector.tensor_tensor(out=var, in0=e2, in1=msq, op=ALU.subtract)
              eps_t = small.tile([50, 1], F32)
              nc.vector.memset(eps_t, float(BN_EPS))
              sdev = small.tile([50, 1], F32)
              nc.scalar.activation(out=sdev, in_=var, func=ACTF.Sqrt,
                                   bias=eps_t, scale=1.0)
              rstd = small.tile([50, 1], F32)
              nc.vector.reciprocal(out=rstd, in_=sdev)
              scl = small.tile([50, 1], F32)
              nc.vector.tensor_tensor(out=scl, in0=rstd, in1=s_ng, op=ALU.mult)
              t2 = small.tile([50, 1], F32)
              nc.vector.tensor_tensor(out=t2, in0=mean, in1=scl, op=ALU.mult)
              bco = small.tile([50, 1], F32)
              nc.vector.tensor_tensor(out=bco, in0=s_beta, in1=t2, op=ALU.subtract)

              h2bn = small.tile([50, 4, 4, BC], BF16)
              nc.vector.tensor_scalar(
                  out=h2bn.rearrange("p i j b -> p (i j b)"), in0=h2f, scalar1=scl,
                  scalar2=bco, op0=ALU.mult, op1=ALU.add)

              # ---- FC1 (+bias+relu) -> x2 [125, 32] x 4 ----
              x2 = []
              with tc.tile_pool(name="fcps", bufs=1, space="PSUM") as fcps:
                  for uc in range(4):
                      pu = fcps.tile([125, BC], F32, tag=f"fc1ps{uc}")
                      for ij in range(16):
                          nc.tensor.matmul(
                              pu, s_w1t[:, uc * 125:(uc + 1) * 125, ij],
                              h2bn[:, ij // 4, ij % 4, :],
                              start=(ij == 0), stop=(ij == 15))
                      xc = small.tile([125, BC], BF16, tag=f"x2_{uc}")
                      nc.scalar.activation(out=xc, in_=pu, func=ACTF.Relu,
                                           bias=s_b1t[:, uc:uc + 1], scale=1.0)
                      x2.append(xc)

                  # ---- FC2 -> logits [10, 32] ----
                  plg = fcps.tile([10, BC], F32, tag="fc2ps")
                  for uc in range(4):
                      nc.tensor.matmul(plg, s_w2t[:, uc, :], x2[uc],
                                       start=(uc == 0), stop=(uc == 3))
                  lg = small.tile([10, BC], F32)
                  nc.vector.tensor_scalar(out=lg, in0=plg, scalar1=s_b2,
                                          scalar2=None, op0=ALU.add)

                  # ---- transpose to [32, 10] + log_softmax ----
                  plt = fcps.tile([BC, 10], F32, tag="ltps")
                  nc.tensor.transpose(plt, lg, s_id10)
                  mx = small.tile([BC, 1], F32)
                  nc.vector.tensor_reduce(out=mx, in_=plt,
                                          axis=mybir.AxisListType.X, op=ALU.max)
                  t1 = small.tile([BC, 10], F32)
                  nc.vector.tensor_scalar(out=t1, in0=plt, scalar1=mx,
                                          scalar2=None, op0=ALU.subtract)
                  ex = small.tile([BC, 10], F32)
                  nc.scalar.activation(out=ex, in_=t1, func=ACTF.Exp)
                  sm = small.tile([BC, 1], F32)
                  nc.vector.tensor_reduce(out=sm, in_=ex,
                                          axis=mybir.AxisListType.X, op=ALU.add)
                  ls = small.tile([BC, 1], F32)
                  nc.scalar.activation(out=ls, in_=sm, func=ACTF.Ln)
                  yt = small.tile([BC, 10], F32)
                  nc.vector.tensor_scalar(out=yt, in0=t1, scalar1=ls,
                                          scalar2=None, op0=ALU.subtract)
                  nc.sync.dma_start(out=d_y[:, :], in_=yt)

    return _patch_nc(nc)


_NC_CACHE = None


def _get_nc():
    global _NC_CACHE
    if _NC_CACHE is None:
        _NC_CACHE = build_nc()
    return _NC_CACHE


def make_in_maps(inputs):
    p = host_prep(inputs)
    shared = {k: p[k] for k in ("w1conv", "negwT", "poswT", "onehot", "z2",
                                "ones_dve", "w1t", "b1t", "w2t", "b2",
                                "neg_gamma", "beta", "ident10")}
    import ml_dtypes
    return [
        {"x_imT": np.ascontiguousarray(
            p["x_im"][c * BC:(c + 1) * BC].T).astype(ml_dtypes.bfloat16),
         **shared}
        for c in range(N_CORES)
    ]


def kernel(**inputs) -> np.ndarray:
    nc = _get_nc()
    in_maps = make_in_maps(inputs)
    res = run_bass_kernel_spmd(nc, in_maps, core_ids=list(range(N_CORES)),
                               trace=False)
    return np.concatenate([res.results[c]["y"] for c in range(N_CORES)],
                          axis=0).astype(np.float32)



# revision 3
# speedup vs baseline: 3635.3801x; 3635.3801x over previous
"""Trainium2 Bass kernel v2 for nn_Net_72447508349482 (dense_cnn).

Key changes vs v1:
  - Adder reduction uses PE column tiling (tile_position=(0,32j)): 4 concurrent
    one-hot reductions in distinct 32-partition column groups -> ~4x less PE
    streaming time. Channel c = 4t+j lives at psum partition 32j + t; the
    permutation is folded into BN/FC1 constants on the host.
  - Coalesced DMAs: 1 conv-rhs DMA per row chunk (4-dim AP from DRAM), 5 patch
    DMAs per k-chunk (overlapping-window AP on SBUF h1), all on the sync queue.
  - h1 kept in bf16 (halves patch DMA bytes, 2x DVE pool maxes).
  - pool2/BN/FC moved out of the adder-psum pool scope so the next rep's conv
    can overlap the AllReduce wait.

conv1 bias folded into adder weights: |p + b - w| = |p - (w - b)|.
Self-contained: includes the BIR multi-wait splitting fix for this container's
walrus build.
"""

import json

import numpy as np

import concourse.bass as bass
import concourse.mybir as mybir
import concourse.tile as tile
from concourse.bass_utils import run_bass_kernel_spmd

N_CORES = 8
B_TOTAL = 256
BC = B_TOTAL // N_CORES          # 32 per core
BN_EPS = 1e-5
F32 = mybir.dt.float32
BF16 = mybir.dt.bfloat16
ALU = mybir.AluOpType
ACTF = mybir.ActivationFunctionType

# channel c = 4t + j  ->  psum partition 32j + t
def _P(c):
    return 32 * (c % 4) + c // 4

# ACT-engine channels (|p-w| via activation Abs); rest on DVE via min identity.
ACT_CH = {4 * t + (t % 4) for t in range(12)} | {48, 49}


# --------------------------------------------------------------------------
# BIR post-processing: split multi-wait instructions (walrus limit = 1).
# --------------------------------------------------------------------------
def _fix_bir_bytes(raw: bytes) -> bytes:
    d = json.loads(raw)
    for fn in d.get("functions", []):
        for b in fn.get("blocks", []):
            insts = b.get("instructions", [])
            i = 0
            while i < len(insts):
                ins = insts[i]
                si = ins.get("sync_info") or {}
                w = si.get("on_wait") or []
                if len(w) > 1:
                    for j, extra in enumerate(w[:-1]):
                        insts.insert(i, {
                            "name": f"{ins['name']}_wsplit{j}",
                            "opcode": "EventSemaphore",
                            "engine": ins["engine"],
                            "ins": [], "outs": [],
                            "debug": ins.get("debug", 0),
                            "sync_info": {"on_update": [], "on_wait": [extra]},
                        })
                        i += 1
                    si["on_wait"] = w[-1:]
                i += 1
    return json.dumps(d).encode()


def _patch_nc(nc):
    cls_fn = type(nc).to_json_bytes
    nc.to_json_bytes = lambda: _fix_bir_bytes(cls_fn(nc))
    return nc


# --------------------------------------------------------------------------
# Host-side input prep.
# --------------------------------------------------------------------------
def host_prep(inputs):
    import ml_dtypes
    bf = ml_dtypes.bfloat16
    f = lambda k: np.ascontiguousarray(np.asarray(inputs[k], np.float32))
    x = f("x")                                       # [256,1,28,28]
    conv1_w, conv1_b, adder_w = f("conv1_w"), f("conv1_b"), f("adder_w")
    p = {}
    p["x_im"] = x.reshape(B_TOTAL, 784)
    wfold = adder_w - conv1_b[None, :, None, None]
    A = (-wfold.reshape(50, 20, 25))                 # [o, c, m] = -w'
    negwT = np.stack(
        [A[:, 5 * q:5 * q + 5, :].transpose(2, 1, 0).reshape(125, 50)
         for q in range(4)], axis=1)                 # [125, 4, 50] = -w'
    p["negwT"] = np.ascontiguousarray(negwT)
    poswT = np.stack(
        [(-A)[:, 5 * q:5 * q + 5, :].transpose(2, 1, 0).reshape(125, 50)
         for q in range(4)], axis=1)                 # [125, 4, 50] = +w'
    p["poswT"] = np.ascontiguousarray(poswT)
    p["w1conv"] = np.ascontiguousarray(conv1_w.reshape(20, 25).T).astype(bf)
    oh = np.zeros((125, 64), bf)
    oh[:, 32] = bf(1.0)
    p["oh1"] = oh
    z2 = np.zeros((125, 64), bf)
    z2[:, 32] = bf(-2.0)
    p["z2"] = z2
    onesdve = np.zeros((125, 128), bf)
    for c in range(50):
        if c not in ACT_CH:
            onesdve[:, _P(c)] = bf(1.0)
    p["ones_dve"] = onesdve
    w1o = f("fc1_w").reshape(500, 50, 16).transpose(1, 0, 2)   # [50, 500, 16]
    w1s = np.zeros((128, 500, 16), np.float32)
    for c in range(50):
        w1s[_P(c)] = w1o[c]
    p["w1t"] = np.ascontiguousarray(w1s).astype(bf)            # [128,500,16]
    p["b1t"] = np.ascontiguousarray(f("fc1_b").reshape(4, 125).T)   # [125,4]
    p["w2t"] = np.ascontiguousarray(
        f("fc2_w").T.reshape(4, 125, 10).transpose(1, 0, 2)).astype(bf)
    p["b2"] = f("fc2_b").reshape(10, 1)
    ng = np.zeros((128, 1), np.float32)
    bt = np.zeros((128, 1), np.float32)
    g, b_ = f("bn_gamma"), f("bn_beta")
    for c in range(50):
        ng[_P(c), 0] = -g[c]
        bt[_P(c), 0] = b_[c]
    p["neg_gamma"] = ng
    p["beta"] = bt
    p["ident10"] = np.eye(10, dtype=np.float32)
    return p


# --------------------------------------------------------------------------
# Device program.
# --------------------------------------------------------------------------
def build_nc(reps: int = 1, for_sim: bool = False):
    nc = bass.Bass("TRN2", target_bir_lowering=False, debug=False,
                   num_devices=1 if for_sim else N_CORES)
    d_x = nc.dram_tensor("x_imT", [784, BC], BF16, kind="ExternalInput")
    d_w1conv = nc.dram_tensor("w1conv", [25, 20], BF16, kind="ExternalInput")
    d_negwT = nc.dram_tensor("negwT", [125, 4, 50], F32, kind="ExternalInput")
    d_poswT = nc.dram_tensor("poswT", [125, 4, 50], F32, kind="ExternalInput")
    d_oh1 = nc.dram_tensor("oh1", [125, 64], BF16, kind="ExternalInput")
    d_z2 = nc.dram_tensor("z2", [125, 64], BF16, kind="ExternalInput")
    d_onesdve = nc.dram_tensor("ones_dve", [125, 128], BF16,
                               kind="ExternalInput")
    d_w1t = nc.dram_tensor("w1t", [128, 500, 16], BF16, kind="ExternalInput")
    d_b1t = nc.dram_tensor("b1t", [125, 4], F32, kind="ExternalInput")
    d_w2t = nc.dram_tensor("w2t", [125, 4, 10], BF16, kind="ExternalInput")
    d_b2 = nc.dram_tensor("b2", [10, 1], F32, kind="ExternalInput")
    d_ng = nc.dram_tensor("neg_gamma", [128, 1], F32, kind="ExternalInput")
    d_beta = nc.dram_tensor("beta", [128, 1], F32, kind="ExternalInput")
    d_id10 = nc.dram_tensor("ident10", [10, 10], F32, kind="ExternalInput")
    d_y = nc.dram_tensor("y", [BC, 10], F32, kind="ExternalOutput")
    cc_in = nc.dram_tensor("cc_in", [128, 2], F32)
    cc_out = nc.dram_tensor("cc_out", [128, 2], F32, addr_space="Shared")

    with tile.TileContext(nc) as tc:
        with (
            tc.tile_pool(name="consts", bufs=1) as consts,
            tc.tile_pool(name="big", bufs=2) as big,
            tc.tile_pool(name="work", bufs=3) as work,
            tc.tile_pool(name="absp", bufs=10) as absp,
            tc.tile_pool(name="small", bufs=2) as small,
        ):
            # ---- load constants ----
            def cload(dram, shape, dt):
                t = consts.tile(shape, dt, tag=f"c_{dram.name}",
                                name=f"c_{dram.name}")
                nc.sync.dma_start(out=t, in_=dram.ap())
                return t
            s_w1conv = cload(d_w1conv, [25, 20], BF16)

            def gload(dram, shape, dt):
                # big constants ride the otherwise-idle gpsimd queue so the
                # conv's sync-queue DMAs aren't stuck behind ~3 MB of loads
                t = consts.tile(shape, dt, tag=f"c_{dram.name}",
                                name=f"c_{dram.name}")
                nc.gpsimd.dma_start(out=t, in_=dram.ap())
                return t
            s_negwT = gload(d_negwT, [125, 4, 50], F32)
            s_poswT = gload(d_poswT, [125, 4, 50], F32)
            s_oh1 = gload(d_oh1, [125, 64], BF16)
            s_z2 = gload(d_z2, [125, 64], BF16)
            s_onesdve = gload(d_onesdve, [125, 128], BF16)
            s_w1t = gload(d_w1t, [128, 500, 16], BF16)
            s_b1t = gload(d_b1t, [125, 4], F32)
            s_w2t = gload(d_w2t, [125, 4, 10], BF16)
            s_b2 = gload(d_b2, [10, 1], F32)
            s_ng = gload(d_ng, [128, 1], F32)
            s_beta = gload(d_beta, [128, 1], F32)
            s_id10 = gload(d_id10, [10, 10], F32)

            x_t = d_x.ap().tensor

            def make_conv(_rep):
              # returns (h1 tile, [12 chunk emitters]); the convps psum pool
              # is entered at chunk 0 and exited after chunk 11, so the whole
              # group can be emitted inside another rep's adder q-loop.
              h1 = big.tile([20, 12, 12, BC], BF16, tag="h1", name="h1")
              holder = {}

              def chunk(ic):
                  def em():
                      if ic == 0:
                          holder["cm"] = tc.tile_pool(name="convps", bufs=1,
                                                      space="PSUM")
                          holder["pool"] = holder["cm"].__enter__()
                      convps = holder["pool"]
                      i0 = 2 * ic
                      rhs = work.tile([25, 2, 24, BC], BF16, tag="convrhs")
                      for r in range(2):
                          src = bass.AP(
                              tensor=x_t,
                              offset=(i0 + r) * 28 * BC,
                              ap=[[28 * BC, 5], [BC, 5], [1, 24 * BC]],
                          )
                          nc.sync.dma_start(out=rhs[:, r], in_=src)
                      ps = convps.tile([20, 2, 24, BC], F32, tag="convps")
                      rflat = rhs.rearrange("p a b c -> p (a b c)")
                      pflat = ps.rearrange("p a b c -> p (a b c)")
                      for s0, s1 in ((0, 512), (512, 1024), (1024, 1536)):
                          nc.tensor.matmul(pflat[:, s0:s1], s_w1conv[:, :],
                                           rflat[:, s0:s1], start=True,
                                           stop=True)
                      codd = work.tile([20, 2, 12, BC], BF16, tag="convodd")
                      pv = ps.rearrange("p a (j two) b -> p a j two b", two=2)
                      if ic % 2 == 0:
                          nc.scalar.activation(out=codd, in_=pv[:, :, :, 1, :],
                                               func=ACTF.Copy)
                      else:
                          nc.vector.tensor_copy(out=codd, in_=pv[:, :, :, 1, :])
                      h1j = work.tile([20, 2, 12, BC], BF16, tag="h1j")
                      nc.vector.tensor_tensor(out=h1j, in0=pv[:, :, :, 0, :],
                                              in1=codd, op=ALU.max)
                      nc.vector.tensor_tensor(out=h1[:, ic], in0=h1j[:, 0],
                                              in1=h1j[:, 1], op=ALU.max)
                      if ic == 11:
                          holder["cm"].__exit__(None, None, None)
                  return em

              return h1, [chunk(ic) for ic in range(12)]

            def emit_adder(h1, next_chunks):
              # ---- adder: elementwise (DVE min / ACT abs) + col-tiled PE;
              # the NEXT rep's conv chunks are interleaved (3 per k-chunk) so
              # its h1 is ready the moment this adder drains. ----
              with tc.tile_pool(name="addps", bufs=1, space="PSUM") as addps:
                  psum_add = addps.tile([128, 2048], F32)
                  for q in range(4):
                      pq = big.tile([125, 8, 8, BC], BF16, tag=f"patches{q}",
                                    name=f"patches{q}")
                      for kh in range(5):
                          for kw in range(5):
                              m = kh * 5 + kw
                              eng = nc.sync if m % 2 == 0 else nc.gpsimd
                              eng.dma_start(
                                  out=pq[m * 5:m * 5 + 5],
                                  in_=h1[5 * q:5 * q + 5, kh:kh + 8,
                                         kw:kw + 8, :],
                              )
                      pqf = pq.rearrange("p i j b -> p (i j b)")
                      for t in range(13):
                          cs = [4 * t + j for j in range(4) if 4 * t + j < 50]
                          ats = []
                          for c in cs:
                              a = absp.tile([125, 2048], BF16, tag="abs")
                              if c in ACT_CH:
                                  nc.scalar.activation(
                                      out=a, in_=pqf, func=ACTF.Abs,
                                      bias=s_negwT[:, q, c:c + 1], scale=1.0)
                              else:
                                  nc.vector.tensor_scalar(
                                      out=a, in0=pqf,
                                      scalar1=s_poswT[:, q, c:c + 1],
                                      scalar2=None, op0=ALU.min)
                              ats.append(a)
                          for nb in range(4):
                              for c, a in zip(cs, ats):
                                  j = c % 4
                                  lhsT = (s_oh1 if c in ACT_CH
                                          else s_z2)[:, 32 - t:64 - t]
                                  nc.tensor.matmul(
                                      psum_add[32 * j:32 * j + 32,
                                               nb * 512:(nb + 1) * 512],
                                      lhsT, a[:, nb * 512:(nb + 1) * 512],
                                      start=(q == 0 and t == 0), stop=False,
                                      tile_position=(0, 32 * j))
                      # sum_k patch into DVE-channel rows (full width)
                      for nb in range(4):
                          nc.tensor.matmul(
                              psum_add[:, nb * 512:(nb + 1) * 512],
                              s_onesdve[:, :],
                              pqf[:, nb * 512:(nb + 1) * 512],
                              start=False, stop=(q == 3))
                      for em in next_chunks[3 * q:3 * q + 3]:
                          em()

                  # ---- pool2 (min) -> h2f [128, 512] ----
                  hs = small.tile([128, 2048], F32, tag="hs")
                  nc.scalar.activation(out=hs, in_=psum_add, func=ACTF.Copy)

              pv2 = hs.rearrange("p (i j two b) -> p i j two b", i=8, j=4,
                                 two=2)
              h2j = small.tile([128, 8, 4, BC], F32, tag="h2j")
              nc.vector.tensor_tensor(out=h2j, in0=pv2[:, :, :, 0, :],
                                      in1=pv2[:, :, :, 1, :], op=ALU.min)
              h2v = h2j.rearrange("p (i two) j b -> p i two j b", two=2)
              h2m = small.tile([128, 4, 4, BC], F32, tag="h2m")
              nc.vector.tensor_tensor(out=h2m, in0=h2v[:, :, 0],
                                      in1=h2v[:, :, 1], op=ALU.min)
              h2f = h2m.rearrange("p i j b -> p (i j b)")      # [128, 512]

              # ---- BN stats + AllReduce ----
              stats = small.tile([128, 2], F32, tag="stats")
              nc.vector.tensor_reduce(out=stats[:, 0:1], in_=h2f,
                                      axis=mybir.AxisListType.X, op=ALU.add)
              junk = small.tile([128, 512], F32, tag="junk")
              nc.vector.tensor_tensor(out=junk, in0=h2f, in1=h2f, op=ALU.mult)
              nc.vector.tensor_reduce(out=stats[:, 1:2], in_=junk,
                                      axis=mybir.AxisListType.X, op=ALU.add)
              nc.gpsimd.dma_start(out=cc_in[:, :], in_=stats)
              if not for_sim:
                  nc.gpsimd.collective_compute(
                      "AllReduce", ALU.add,
                      replica_groups=[list(range(N_CORES))],
                      ins=[cc_in.ap().opt()], outs=[cc_out.ap().opt()])
              return h2m

            def emit_tail(h2m):
              h2f = h2m.rearrange("p i j b -> p (i j b)")      # [128, 512]
              gs = small.tile([128, 2], F32, tag="gs")
              nc.gpsimd.dma_start(out=gs, in_=(cc_in if for_sim else cc_out)[:, :])

              # ---- BN affine coefficients ----
              inv_n = 1.0 / (B_TOTAL * 16)
              mean = small.tile([128, 1], F32, tag="mean")
              nc.vector.tensor_scalar(out=mean, in0=gs[:, 0:1], scalar1=inv_n,
                                      scalar2=None, op0=ALU.mult)
              e2 = small.tile([128, 1], F32, tag="e2")
              nc.vector.tensor_scalar(out=e2, in0=gs[:, 1:2], scalar1=inv_n,
                                      scalar2=None, op0=ALU.mult)
              msq = small.tile([128, 1], F32, tag="msq")
              nc.vector.tensor_tensor(out=msq, in0=mean, in1=mean, op=ALU.mult)
              var = small.tile([128, 1], F32, tag="var")
              nc.vector.tensor_tensor(out=var, in0=e2, in1=msq,
                                      op=ALU.subtract)
              eps_t = small.tile([128, 1], F32, tag="eps")
              nc.vector.memset(eps_t, float(BN_EPS))
              sdev = small.tile([128, 1], F32, tag="sdev")
              nc.scalar.activation(out=sdev, in_=var, func=ACTF.Sqrt,
                                   bias=eps_t, scale=1.0)
              rstd = small.tile([128, 1], F32, tag="rstd")
              nc.vector.reciprocal(out=rstd, in_=sdev)
              scl = small.tile([128, 1], F32, tag="scl")
              nc.vector.tensor_tensor(out=scl, in0=rstd, in1=s_ng, op=ALU.mult)
              t2 = small.tile([128, 1], F32, tag="t2")
              nc.vector.tensor_tensor(out=t2, in0=mean, in1=scl, op=ALU.mult)
              bco = small.tile([128, 1], F32, tag="bco")
              nc.vector.tensor_tensor(out=bco, in0=s_beta, in1=t2,
                                      op=ALU.subtract)

              h2bn = small.tile([128, 4, 4, BC], BF16, tag="h2bn")
              nc.vector.tensor_scalar(
                  out=h2bn.rearrange("p i j b -> p (i j b)"), in0=h2f,
                  scalar1=scl, scalar2=bco, op0=ALU.mult, op1=ALU.add)

              # ---- FC1 (+bias+relu) -> x2 [125, 32] x 4 ----
              x2 = []
              with tc.tile_pool(name="fcps", bufs=1, space="PSUM") as fcps:
                  fcbig = fcps.tile([125, 4, BC], F32, tag="fc1ps")
                  for uc in range(4):
                      pu = fcbig[:, uc, :]
                      for ij in range(16):
                          nc.tensor.matmul(
                              pu, s_w1t[:, uc * 125:(uc + 1) * 125, ij],
                              h2bn[:, ij // 4, ij % 4, :],
                              start=(ij == 0), stop=(ij == 15))
                      xc = small.tile([125, BC], BF16, tag=f"x2_{uc}")
                      nc.scalar.activation(out=xc, in_=pu, func=ACTF.Relu,
                                           bias=s_b1t[:, uc:uc + 1], scale=1.0)
                      x2.append(xc)

                  # ---- FC2 -> logits [10, 32] ----
                  plg = fcps.tile([10, BC], F32, tag="fc2ps")
                  for uc in range(4):
                      nc.tensor.matmul(plg, s_w2t[:, uc, :], x2[uc],
                                       start=(uc == 0), stop=(uc == 3))
                  lg = small.tile([10, BC], F32, tag="lg")
                  nc.vector.tensor_scalar(out=lg, in0=plg, scalar1=s_b2,
                                          scalar2=None, op0=ALU.add)

                  # ---- transpose to [32, 10] + log_softmax ----
                  plt = fcps.tile([BC, 10], F32, tag="ltps")
                  nc.tensor.transpose(plt, lg, s_id10)
                  mx = small.tile([BC, 1], F32, tag="mx")
                  nc.vector.tensor_reduce(out=mx, in_=plt,
                                          axis=mybir.AxisListType.X, op=ALU.max)
                  t1 = small.tile([BC, 10], F32, tag="t1")
                  nc.vector.tensor_scalar(out=t1, in0=plt, scalar1=mx,
                                          scalar2=None, op0=ALU.subtract)
                  ex = small.tile([BC, 10], F32, tag="ex")
                  nc.scalar.activation(out=ex, in_=t1, func=ACTF.Exp)
                  sm = small.tile([BC, 1], F32, tag="sm")
                  nc.vector.tensor_reduce(out=sm, in_=ex,
                                          axis=mybir.AxisListType.X, op=ALU.add)
                  ls = small.tile([BC, 1], F32, tag="ls")
                  nc.scalar.activation(out=ls, in_=sm, func=ACTF.Ln)
                  yt = small.tile([BC, 10], F32, tag="yt")
                  nc.vector.tensor_scalar(out=yt, in0=t1, scalar1=ls,
                                          scalar2=None, op0=ALU.subtract)
                  nc.gpsimd.dma_start(out=d_y[:, :], in_=yt)

            # Software-pipelined emission:
            #   conv(0); [adder(i) + interleaved conv(i+1)]; tail(i-1) after
            #   adder(i) so no engine queue ever stalls on the AllReduce.
            cur_h1, cur_chunks = make_conv(0)
            for em in cur_chunks:
                em()
            pending = None
            for i in range(reps):
                if i + 1 < reps:
                    nxt_h1, nxt_chunks = make_conv(i + 1)
                else:
                    nxt_h1, nxt_chunks = None, [lambda: None] * 12
                h2m = emit_adder(cur_h1, nxt_chunks)
                if pending is not None:
                    emit_tail(pending)
                pending = h2m
                cur_h1 = nxt_h1
            emit_tail(pending)

    return _patch_nc(nc)


_NC_CACHE = None


def _get_nc():
    global _NC_CACHE
    if _NC_CACHE is None:
        _NC_CACHE = build_nc()
    return _NC_CACHE


def make_in_maps(inputs):
    p = host_prep(inputs)
    shared = {k: p[k] for k in ("w1conv", "negwT", "poswT", "oh1", "z2",
                                "ones_dve", "w1t", "b1t", "w2t", "b2",
                                "neg_gamma", "beta", "ident10")}
    import ml_dtypes
    return [
        {"x_imT": np.ascontiguousarray(
            p["x_im"][c * BC:(c + 1) * BC].T).astype(ml_dtypes.bfloat16),
         **shared}
        for c in range(N_CORES)
    ]


def kernel(**inputs) -> np.ndarray:
    nc = _get_nc()
    in_maps = make_in_maps(inputs)
    res = run_bass_kernel_spmd(nc, in_maps, core_ids=list(range(N_CORES)),
                               trace=False)
    return np.concatenate([res.results[c]["y"] for c in range(N_CORES)],
                          axis=0).astype(np.float32)


# revision 4
# speedup vs baseline: 4145.7058x; 1.1404x over previous
"""Trainium2 Bass kernel v2 for nn_Net_72447508349482 (dense_cnn).

Key changes vs v1:
  - Adder reduction uses PE column tiling (tile_position=(0,32j)): 4 concurrent
    one-hot reductions in distinct 32-partition column groups -> ~4x less PE
    streaming time. Channel c = 4t+j lives at psum partition 32j + t; the
    permutation is folded into BN/FC1 constants on the host.
  - Coalesced DMAs: 1 conv-rhs DMA per row chunk (4-dim AP from DRAM), 5 patch
    DMAs per k-chunk (overlapping-window AP on SBUF h1), all on the sync queue.
  - h1 kept in bf16 (halves patch DMA bytes, 2x DVE pool maxes).
  - pool2/BN/FC moved out of the adder-psum pool scope so the next rep's conv
    can overlap the AllReduce wait.

conv1 bias folded into adder weights: |p + b - w| = |p - (w - b)|.
Self-contained: includes the BIR multi-wait splitting fix for this container's
walrus build.
"""

import json

import numpy as np

import concourse.bass as bass
import concourse.mybir as mybir
import concourse.tile as tile
from concourse.bass_utils import run_bass_kernel_spmd

N_CORES = 8
B_TOTAL = 256
BC = B_TOTAL // N_CORES          # 32 per core
BN_EPS = 1e-5
F32 = mybir.dt.float32
BF16 = mybir.dt.bfloat16
ALU = mybir.AluOpType
ACTF = mybir.ActivationFunctionType

# channel c = 4t + j  ->  psum partition 32j + t
def _P(c):
    return 32 * (c % 4) + c // 4

# ACT-engine channels (|p-w| via activation Abs); rest on DVE via min identity.
ACT_CH = {4 * t + (t % 4) for t in range(12)} | {48, 49}


# --------------------------------------------------------------------------
# BIR post-processing: split multi-wait instructions (walrus limit = 1).
# --------------------------------------------------------------------------
def _fix_bir_bytes(raw: bytes) -> bytes:
    d = json.loads(raw)
    for fn in d.get("functions", []):
        for b in fn.get("blocks", []):
            insts = b.get("instructions", [])
            i = 0
            while i < len(insts):
                ins = insts[i]
                si = ins.get("sync_info") or {}
                w = si.get("on_wait") or []
                if len(w) > 1:
                    for j, extra in enumerate(w[:-1]):
                        insts.insert(i, {
                            "name": f"{ins['name']}_wsplit{j}",
                            "opcode": "EventSemaphore",
                            "engine": ins["engine"],
                            "ins": [], "outs": [],
                            "debug": ins.get("debug", 0),
                            "sync_info": {"on_update": [], "on_wait": [extra]},
                        })
                        i += 1
                    si["on_wait"] = w[-1:]
                i += 1
    return json.dumps(d).encode()


def _patch_nc(nc):
    cls_fn = type(nc).to_json_bytes
    nc.to_json_bytes = lambda: _fix_bir_bytes(cls_fn(nc))
    return nc


# --------------------------------------------------------------------------
# Host-side input prep.
# --------------------------------------------------------------------------
def host_prep(inputs):
    import ml_dtypes
    bf = ml_dtypes.bfloat16
    f = lambda k: np.ascontiguousarray(np.asarray(inputs[k], np.float32))
    x = f("x")                                       # [256,1,28,28]
    conv1_w, conv1_b, adder_w = f("conv1_w"), f("conv1_b"), f("adder_w")
    p = {}
    p["x_im"] = x.reshape(B_TOTAL, 784)
    wfold = adder_w - conv1_b[None, :, None, None]
    A = (-wfold.reshape(50, 20, 25))                 # [o, c, m] = -w'
    negwT = np.stack(
        [A[:, 5 * q:5 * q + 5, :].transpose(2, 1, 0).reshape(125, 50)
         for q in range(4)], axis=1)                 # [125, 4, 50] = -w'
    p["negwT"] = np.ascontiguousarray(negwT)
    poswT = np.stack(
        [(-A)[:, 5 * q:5 * q + 5, :].transpose(2, 1, 0).reshape(125, 50)
         for q in range(4)], axis=1)                 # [125, 4, 50] = +w'
    p["poswT"] = np.ascontiguousarray(poswT)
    p["w1conv"] = np.ascontiguousarray(conv1_w.reshape(20, 25).T).astype(bf)
    oh = np.zeros((125, 64), bf)
    oh[:, 32] = bf(1.0)
    p["oh1"] = oh
    z2 = np.zeros((125, 64), bf)
    z2[:, 32] = bf(-2.0)
    p["z2"] = z2
    onesdve = np.zeros((125, 128), bf)
    for c in range(50):
        if c not in ACT_CH:
            onesdve[:, _P(c)] = bf(1.0)
    p["ones_dve"] = onesdve
    w1o = f("fc1_w").reshape(500, 50, 16).transpose(1, 0, 2)   # [50, 500, 16]
    w1s = np.zeros((128, 500, 16), np.float32)
    for c in range(50):
        w1s[_P(c)] = w1o[c]
    p["w1t"] = np.ascontiguousarray(w1s).astype(bf)            # [128,500,16]
    p["b1t"] = np.ascontiguousarray(f("fc1_b").reshape(4, 125).T)   # [125,4]
    p["w2t"] = np.ascontiguousarray(
        f("fc2_w").T.reshape(4, 125, 10).transpose(1, 0, 2)).astype(bf)
    p["b2"] = f("fc2_b").reshape(10, 1)
    ng = np.zeros((128, 1), np.float32)
    bt = np.zeros((128, 1), np.float32)
    g, b_ = f("bn_gamma"), f("bn_beta")
    for c in range(50):
        ng[_P(c), 0] = -g[c]
        bt[_P(c), 0] = b_[c]
    p["neg_gamma"] = ng
    p["beta"] = bt
    p["ident10"] = np.eye(10, dtype=np.float32)
    return p


# --------------------------------------------------------------------------
# Device program.
# --------------------------------------------------------------------------
def build_nc(reps: int = 1, for_sim: bool = False):
    nc = bass.Bass("TRN2", target_bir_lowering=False, debug=False,
                   num_devices=1 if for_sim else N_CORES)
    d_x = nc.dram_tensor("x_imT", [784, BC], BF16, kind="ExternalInput")
    d_w1conv = nc.dram_tensor("w1conv", [25, 20], BF16, kind="ExternalInput")
    d_negwT = nc.dram_tensor("negwT", [125, 4, 50], F32, kind="ExternalInput")
    d_poswT = nc.dram_tensor("poswT", [125, 4, 50], F32, kind="ExternalInput")
    d_oh1 = nc.dram_tensor("oh1", [125, 64], BF16, kind="ExternalInput")
    d_z2 = nc.dram_tensor("z2", [125, 64], BF16, kind="ExternalInput")
    d_onesdve = nc.dram_tensor("ones_dve", [125, 128], BF16,
                               kind="ExternalInput")
    d_w1t = nc.dram_tensor("w1t", [128, 500, 16], BF16, kind="ExternalInput")
    d_b1t = nc.dram_tensor("b1t", [125, 4], F32, kind="ExternalInput")
    d_w2t = nc.dram_tensor("w2t", [125, 4, 10], BF16, kind="ExternalInput")
    d_b2 = nc.dram_tensor("b2", [10, 1], F32, kind="ExternalInput")
    d_ng = nc.dram_tensor("neg_gamma", [128, 1], F32, kind="ExternalInput")
    d_beta = nc.dram_tensor("beta", [128, 1], F32, kind="ExternalInput")
    d_id10 = nc.dram_tensor("ident10", [10, 10], F32, kind="ExternalInput")
    d_y = nc.dram_tensor("y", [BC, 10], F32, kind="ExternalOutput")
    cc_in = nc.dram_tensor("cc_in", [128, 2], F32)
    cc_out = nc.dram_tensor("cc_out", [128, 2], F32, addr_space="Shared")

    with tile.TileContext(nc) as tc:
        with (
            tc.tile_pool(name="consts", bufs=1) as consts,
            tc.tile_pool(name="big", bufs=2) as big,
            tc.tile_pool(name="work", bufs=3) as work,
            tc.tile_pool(name="absp", bufs=14) as absp,
            tc.tile_pool(name="small", bufs=2) as small,
        ):
            # ---- load constants ----
            def cload(dram, shape, dt):
                t = consts.tile(shape, dt, tag=f"c_{dram.name}",
                                name=f"c_{dram.name}")
                nc.sync.dma_start(out=t, in_=dram.ap())
                return t
            s_w1conv = cload(d_w1conv, [25, 20], BF16)

            def gload(dram, shape, dt):
                # big constants ride the otherwise-idle gpsimd queue so the
                # conv's sync-queue DMAs aren't stuck behind ~3 MB of loads
                t = consts.tile(shape, dt, tag=f"c_{dram.name}",
                                name=f"c_{dram.name}")
                nc.gpsimd.dma_start(out=t, in_=dram.ap())
                return t
            s_negwT = gload(d_negwT, [125, 4, 50], F32)
            s_poswT = gload(d_poswT, [125, 4, 50], F32)
            s_oh1 = gload(d_oh1, [125, 64], BF16)
            s_z2 = gload(d_z2, [125, 64], BF16)
            s_onesdve = gload(d_onesdve, [125, 128], BF16)
            s_w1t = gload(d_w1t, [128, 500, 16], BF16)
            s_b1t = gload(d_b1t, [125, 4], F32)
            s_w2t = gload(d_w2t, [125, 4, 10], BF16)
            s_b2 = gload(d_b2, [10, 1], F32)
            s_ng = gload(d_ng, [128, 1], F32)
            s_beta = gload(d_beta, [128, 1], F32)
            s_id10 = gload(d_id10, [10, 10], F32)

            x_t = d_x.ap().tensor

            def make_conv(_rep):
              # returns (h1 tile, [12 chunk emitters]); the convps psum pool
              # is entered at chunk 0 and exited after chunk 11, so the whole
              # group can be emitted inside another rep's adder q-loop.
              h1 = big.tile([20, 12, 12, BC], BF16, tag="h1", name="h1")
              holder = {}

              def chunk(ic):
                  def em():
                      if ic == 0:
                          holder["cm"] = tc.tile_pool(name="convps", bufs=1,
                                                      space="PSUM")
                          holder["pool"] = holder["cm"].__enter__()
                      convps = holder["pool"]
                      i0 = 2 * ic
                      rhs = work.tile([25, 2, 24, BC], BF16, tag="convrhs")
                      for r in range(2):
                          src = bass.AP(
                              tensor=x_t,
                              offset=(i0 + r) * 28 * BC,
                              ap=[[28 * BC, 5], [BC, 5], [1, 24 * BC]],
                          )
                          nc.sync.dma_start(out=rhs[:, r], in_=src)
                      ps = convps.tile([20, 2, 24, BC], F32, tag="convps")
                      rflat = rhs.rearrange("p a b c -> p (a b c)")
                      pflat = ps.rearrange("p a b c -> p (a b c)")
                      for s0, s1 in ((0, 512), (512, 1024), (1024, 1536)):
                          nc.tensor.matmul(pflat[:, s0:s1], s_w1conv[:, :],
                                           rflat[:, s0:s1], start=True,
                                           stop=True)
                      codd = work.tile([20, 2, 12, BC], BF16, tag="convodd")
                      pv = ps.rearrange("p a (j two) b -> p a j two b", two=2)
                      if ic % 2 == 0:
                          nc.scalar.activation(out=codd, in_=pv[:, :, :, 1, :],
                                               func=ACTF.Copy)
                      else:
                          nc.vector.tensor_copy(out=codd, in_=pv[:, :, :, 1, :])
                      h1j = work.tile([20, 2, 12, BC], BF16, tag="h1j")
                      nc.vector.tensor_tensor(out=h1j, in0=pv[:, :, :, 0, :],
                                              in1=codd, op=ALU.max)
                      nc.vector.tensor_tensor(out=h1[:, ic], in0=h1j[:, 0],
                                              in1=h1j[:, 1], op=ALU.max)
                      if ic == 11:
                          holder["cm"].__exit__(None, None, None)
                  return em

              return h1, [chunk(ic) for ic in range(12)]

            def emit_patches(h1, qs):
              pqs = {}
              for q in qs:
                  pq = big.tile([125, 8, 8, BC], BF16, tag=f"patches{q}",
                                name=f"patches{q}")
                  for kh in range(5):
                      for kw in range(5):
                          m = kh * 5 + kw
                          eng = nc.sync if m % 2 == 0 else nc.gpsimd
                          eng.dma_start(
                              out=pq[m * 5:m * 5 + 5],
                              in_=h1[5 * q:5 * q + 5, kh:kh + 8, kw:kw + 8, :],
                          )
                  pqs[q] = pq
              return pqs

            def emit_adder(h1, pq0, next_h1, next_chunks, tail_cb):
              # ---- adder: elementwise (DVE min / ACT abs) + col-tiled PE.
              # Interleaved into this rep's adder: the NEXT rep's conv chunks
              # (3 per k-chunk), the PREVIOUS rep's CC-dependent tail (after
              # the q==1 section, when its AllReduce has long completed), and
              # the NEXT rep's q0 patch DMAs -- so no engine queue idles at
              # the rep boundary. ----
              with tc.tile_pool(name="addps", bufs=1, space="PSUM") as addps:
                  pqs = emit_patches(h1, [1, 2, 3])
                  pqs[0] = pq0
                  psum_add = addps.tile([128, 2048], F32)
                  for q in range(4):
                      pqf = pqs[q].rearrange("p i j b -> p (i j b)")
                      for t in range(13):
                          cs = [4 * t + j for j in range(4) if 4 * t + j < 50]
                          ats = []
                          for c in cs:
                              a = absp.tile([125, 2048], BF16, tag="abs")
                              if c in ACT_CH:
                                  nc.scalar.activation(
                                      out=a, in_=pqf, func=ACTF.Abs,
                                      bias=s_negwT[:, q, c:c + 1], scale=1.0)
                              else:
                                  nc.vector.tensor_scalar(
                                      out=a, in0=pqf,
                                      scalar1=s_poswT[:, q, c:c + 1],
                                      scalar2=None, op0=ALU.min)
                              ats.append(a)
                          for nb in range(4):
                              for c, a in zip(cs, ats):
                                  j = c % 4
                                  lhsT = (s_oh1 if c in ACT_CH
                                          else s_z2)[:, 32 - t:64 - t]
                                  nc.tensor.matmul(
                                      psum_add[32 * j:32 * j + 32,
                                               nb * 512:(nb + 1) * 512],
                                      lhsT, a[:, nb * 512:(nb + 1) * 512],
                                      start=(q == 0 and t == 0), stop=False,
                                      tile_position=(0, 32 * j))
                      # sum_k patch into DVE-channel rows (full width)
                      for nb in range(4):
                          nc.tensor.matmul(
                              psum_add[:, nb * 512:(nb + 1) * 512],
                              s_onesdve[:, :],
                              pqf[:, nb * 512:(nb + 1) * 512],
                              start=False, stop=(q == 3))
                      for em in next_chunks[3 * q:3 * q + 3]:
                          em()
                      if q == 1 and tail_cb is not None:
                          tail_cb()
                  next_pq0 = (emit_patches(next_h1, [0])[0]
                              if next_h1 is not None else None)

                  # ---- pool2 (min) -> h2f [128, 512] ----
                  hs = small.tile([128, 2048], F32, tag="hs")
                  nc.scalar.activation(out=hs, in_=psum_add, func=ACTF.Copy)

              pv2 = hs.rearrange("p (i j two b) -> p i j two b", i=8, j=4,
                                 two=2)
              h2j = small.tile([128, 8, 4, BC], F32, tag="h2j")
              nc.vector.tensor_tensor(out=h2j, in0=pv2[:, :, :, 0, :],
                                      in1=pv2[:, :, :, 1, :], op=ALU.min)
              h2v = h2j.rearrange("p (i two) j b -> p i two j b", two=2)
              h2m = small.tile([128, 4, 4, BC], F32, tag="h2m")
              nc.vector.tensor_tensor(out=h2m, in0=h2v[:, :, 0],
                                      in1=h2v[:, :, 1], op=ALU.min)
              h2f = h2m.rearrange("p i j b -> p (i j b)")      # [128, 512]

              # ---- BN stats + AllReduce ----
              stats = small.tile([128, 2], F32, tag="stats")
              nc.vector.tensor_reduce(out=stats[:, 0:1], in_=h2f,
                                      axis=mybir.AxisListType.X, op=ALU.add)
              junk = small.tile([128, 512], F32, tag="junk")
              nc.vector.tensor_tensor(out=junk, in0=h2f, in1=h2f, op=ALU.mult)
              nc.vector.tensor_reduce(out=stats[:, 1:2], in_=junk,
                                      axis=mybir.AxisListType.X, op=ALU.add)
              nc.gpsimd.dma_start(out=cc_in[:, :], in_=stats)
              if not for_sim:
                  nc.gpsimd.collective_compute(
                      "AllReduce", ALU.add,
                      replica_groups=[list(range(N_CORES))],
                      ins=[cc_in.ap().opt()], outs=[cc_out.ap().opt()])
              return h2m, next_pq0

            def emit_tail(h2m):
              h2f = h2m.rearrange("p i j b -> p (i j b)")      # [128, 512]
              gs = small.tile([128, 2], F32, tag="gs")
              nc.gpsimd.dma_start(out=gs, in_=(cc_in if for_sim else cc_out)[:, :])

              # ---- BN affine coefficients ----
              inv_n = 1.0 / (B_TOTAL * 16)
              mean = small.tile([128, 1], F32, tag="mean")
              nc.vector.tensor_scalar(out=mean, in0=gs[:, 0:1], scalar1=inv_n,
                                      scalar2=None, op0=ALU.mult)
              e2 = small.tile([128, 1], F32, tag="e2")
              nc.vector.tensor_scalar(out=e2, in0=gs[:, 1:2], scalar1=inv_n,
                                      scalar2=None, op0=ALU.mult)
              msq = small.tile([128, 1], F32, tag="msq")
              nc.vector.tensor_tensor(out=msq, in0=mean, in1=mean, op=ALU.mult)
              var = small.tile([128, 1], F32, tag="var")
              nc.vector.tensor_tensor(out=var, in0=e2, in1=msq,
                                      op=ALU.subtract)
              eps_t = small.tile([128, 1], F32, tag="eps")
              nc.vector.memset(eps_t, float(BN_EPS))
              sdev = small.tile([128, 1], F32, tag="sdev")
              nc.scalar.activation(out=sdev, in_=var, func=ACTF.Sqrt,
                                   bias=eps_t, scale=1.0)
              rstd = small.tile([128, 1], F32, tag="rstd")
              nc.vector.reciprocal(out=rstd, in_=sdev)
              scl = small.tile([128, 1], F32, tag="scl")
              nc.vector.tensor_tensor(out=scl, in0=rstd, in1=s_ng, op=ALU.mult)
              t2 = small.tile([128, 1], F32, tag="t2")
              nc.vector.tensor_tensor(out=t2, in0=mean, in1=scl, op=ALU.mult)
              bco = small.tile([128, 1], F32, tag="bco")
              nc.vector.tensor_tensor(out=bco, in0=s_beta, in1=t2,
                                      op=ALU.subtract)

              h2bn = small.tile([128, 4, 4, BC], BF16, tag="h2bn")
              nc.vector.tensor_scalar(
                  out=h2bn.rearrange("p i j b -> p (i j b)"), in0=h2f,
                  scalar1=scl, scalar2=bco, op0=ALU.mult, op1=ALU.add)

              # ---- FC1 (+bias+relu) -> x2 [125, 32] x 4 ----
              x2 = []
              with tc.tile_pool(name="fcps", bufs=1, space="PSUM") as fcps:
                  # single 1-bank psum tile for FC1(4x32) + FC2(32) + the
                  # transposed logits (16) so addps(4) + convps(3) + fcps(1)
                  # fit the 8 PSUM banks with everything overlapped
                  fcall = fcps.tile([128, 176], F32, tag="fcall")
                  fcbig = fcall[0:125, 0:128].rearrange(
                      "p (u b) -> p u b", u=4)
                  for uc in range(4):
                      pu = fcbig[:, uc, :]
                      for ij in range(16):
                          nc.tensor.matmul(
                              pu, s_w1t[:, uc * 125:(uc + 1) * 125, ij],
                              h2bn[:, ij // 4, ij % 4, :],
                              start=(ij == 0), stop=(ij == 15))
                      xc = small.tile([125, BC], BF16, tag=f"x2_{uc}")
                      nc.scalar.activation(out=xc, in_=pu, func=ACTF.Relu,
                                           bias=s_b1t[:, uc:uc + 1], scale=1.0)
                      x2.append(xc)

                  # ---- FC2 -> logits [10, 32] ----
                  plg = fcall[0:10, 128:160]
                  for uc in range(4):
                      nc.tensor.matmul(plg, s_w2t[:, uc, :], x2[uc],
                                       start=(uc == 0), stop=(uc == 3))
                  lg = small.tile([10, BC], F32, tag="lg")
                  nc.vector.tensor_scalar(out=lg, in0=plg, scalar1=s_b2,
                                          scalar2=None, op0=ALU.add)

                  # ---- transpose to [32, 10] + log_softmax ----
                  plt = fcall[0:BC, 160:170]
                  nc.tensor.transpose(plt, lg, s_id10)
                  mx = small.tile([BC, 1], F32, tag="mx")
                  nc.vector.tensor_reduce(out=mx, in_=plt,
                                          axis=mybir.AxisListType.X, op=ALU.max)
                  t1 = small.tile([BC, 10], F32, tag="t1")
                  nc.vector.tensor_scalar(out=t1, in0=plt, scalar1=mx,
                                          scalar2=None, op0=ALU.subtract)
                  ex = small.tile([BC, 10], F32, tag="ex")
                  nc.scalar.activation(out=ex, in_=t1, func=ACTF.Exp)
                  sm = small.tile([BC, 1], F32, tag="sm")
                  nc.vector.tensor_reduce(out=sm, in_=ex,
                                          axis=mybir.AxisListType.X, op=ALU.add)
                  ls = small.tile([BC, 1], F32, tag="ls")
                  nc.scalar.activation(out=ls, in_=sm, func=ACTF.Ln)
                  yt = small.tile([BC, 10], F32, tag="yt")
                  nc.vector.tensor_scalar(out=yt, in0=t1, scalar1=ls,
                                          scalar2=None, op0=ALU.subtract)
                  nc.gpsimd.dma_start(out=d_y[:, :], in_=yt)

            # Software-pipelined emission:
            #   conv(0); [adder(i) + interleaved conv(i+1)]; tail(i-1) after
            #   adder(i) so no engine queue ever stalls on the AllReduce.
            cur_h1, cur_chunks = make_conv(0)
            for em in cur_chunks:
                em()
            cur_pq0 = emit_patches(cur_h1, [0])[0]
            pending = None
            for i in range(reps):
                if i + 1 < reps:
                    nxt_h1, nxt_chunks = make_conv(i + 1)
                else:
                    nxt_h1, nxt_chunks = None, [lambda: None] * 12
                tail_cb = ((lambda p=pending: emit_tail(p))
                           if pending is not None else None)
                h2m, nxt_pq0 = emit_adder(cur_h1, cur_pq0, nxt_h1,
                                          nxt_chunks, tail_cb)
                pending = h2m
                cur_h1, cur_pq0 = nxt_h1, nxt_pq0
            emit_tail(pending)

    return _patch_nc(nc)


_NC_CACHE = None


def _get_nc():
    global _NC_CACHE
    if _NC_CACHE is None:
        _NC_CACHE = build_nc()
    return _NC_CACHE


def make_in_maps(inputs):
    p = host_prep(inputs)
    shared = {k: p[k] for k in ("w1conv", "negwT", "poswT", "oh1", "z2",
                                "ones_dve", "w1t", "b1t", "w2t", "b2",
                                "neg_gamma", "beta", "ident10")}
    import ml_dtypes
    return [
        {"x_imT": np.ascontiguousarray(
            p["x_im"][c * BC:(c + 1) * BC].T).astype(ml_dtypes.bfloat16),
         **shared}
        for c in range(N_CORES)
    ]


def kernel(**inputs) -> np.ndarray:
    nc = _get_nc()
    in_maps = make_in_maps(inputs)
    res = run_bass_kernel_spmd(nc, in_maps, core_ids=list(range(N_CORES)),
                               trace=False)
    return np.concatenate([res.results[c]["y"] for c in range(N_CORES)],
                          axis=0).astype(np.float32)
